# revision 1
# baseline (speedup 1.0000x reference)
"""Trainium2 Bass kernel for nn_BucketedGoWatti (sparse windowed attention pooling).

Math (B=4, L=4096, T=32, DH=1024, DG=256, DP=256, WIN=1024, STRIDE=256, W=13):
  q  = G @ Wq_core;  k = H @ Wk_core (window-independent)
  logits[b,w,t,l] = slice of global  s * (q @ Wk_core^T) @ H^T
  alpha = softmax in window; Zw[b,t,w,:] = alpha @ Hw
  wlog[b,t,w] = Zw . qw2,  qw2 = (G@Wq_win) @ Wk_win^T * DH^-0.5
  Z = softmax_w(wlog) @ Zw   (tiny; done on host at gather time)

Sharding: core c -> batch b=c//2, window half c%2 (even: windows 0-6 over
l in [0,2560); odd: windows 6-12 over l in [1536,4096); window 6 duplicated
so all 8 cores run one SPMD program shape). Cross-window combine on host.

Precision: big matmuls in float32r (~1.5e-4 rel); qw2 path bf16 (negligible
through the 13-way combine softmax); softmax/normalization fp32.
"""
import numpy as np
import ml_dtypes
from contextlib import ExitStack

import concourse.bacc as bacc
import concourse.tile as tile
import concourse.mybir as mybir
import concourse.masks as masks
from concourse.bass_utils import run_bass_kernel_spmd

F32 = mybir.dt.float32
F32R = mybir.dt.float32r
BF16 = mybir.dt.bfloat16
ActFn = mybir.ActivationFunctionType
Alu = mybir.AluOpType

B, L, T = 4, 4096, 32
DH, DG, DP = 1024, 256, 256
WIN, STRIDE = 1024, 256
W = (L - WIN) // STRIDE + 1          # 13
SPAN = 2560                          # per-core l-span
NLT = SPAN // 128                    # 20 l-tiles
NCH = SPAN // 256                    # 10 logits chunks of 256
WLOC = 7                             # windows per core
NDT = DH // 128                      # 8 d-tiles
S_CORE = 1.0 / float(np.sqrt(DP))
S_WIN = 1.0 / float(np.sqrt(DH))

_CACHE = {}


def _build(with_mask: bool, stage: int = 99):
    nc = bacc.Bacc("TRN2", debug=False, target_bir_lowering=False)

    Hn_d = nc.dram_tensor("Hn", [SPAN, DH], F32R, kind="ExternalInput")
    HT_d = nc.dram_tensor("HT", [DH, SPAN], F32R, kind="ExternalInput")
    GT_d = nc.dram_tensor("GT", [DG, T], F32R, kind="ExternalInput")
    Wqc_d = nc.dram_tensor("Wqc", [DG, DP], F32R, kind="ExternalInput")
    WkcT_d = nc.dram_tensor("WkcT", [DP, DH], F32R, kind="ExternalInput")
    Wqw_d = nc.dram_tensor("Wqw", [DG, DH], F32R, kind="ExternalInput")
    WkwT_d = nc.dram_tensor("WkwT", [DH, DH], BF16, kind="ExternalInput")
    if with_mask:
        mb_d = nc.dram_tensor("maskbias", [1, SPAN], F32R, kind="ExternalInput")
        ones_d = nc.dram_tensor("onesrow", [1, T], F32R, kind="ExternalInput")
    zw_d = nc.dram_tensor("Zw_out", [WLOC * T, DH], F32, kind="ExternalOutput")
    wl_d = nc.dram_tensor("wlog_out", [T, WLOC], F32, kind="ExternalOutput")

    with tile.TileContext(nc) as tc, ExitStack() as ctx:
        const = ctx.enter_context(tc.tile_pool(name="const", bufs=1))
        hpool = ctx.enter_context(tc.tile_pool(name="hpool", bufs=16))
        htp = ctx.enter_context(tc.tile_pool(name="htp", bufs=12))
        sb = ctx.enter_context(tc.tile_pool(name="sb", bufs=1))
        sexp = ctx.enter_context(tc.tile_pool(name="sexp", bufs=1))
        pj = ctx.enter_context(tc.tile_pool(name="pj", bufs=2, space="PSUM"))
        lg = ctx.enter_context(tc.tile_pool(name="lg", bufs=2, space="PSUM"))
        zp = ctx.enter_context(tc.tile_pool(name="zp", bufs=4, space="PSUM"))

        # ---- small resident inputs ----
        ident = const.tile([128, 128], F32, tag="ident")
        masks.make_identity(nc, ident[:])
        gt = const.tile([128, 2 * T], F32R, tag="gt")
        wqc = const.tile([128, 2 * DP], F32R, tag="wqc")
        wkcT = const.tile([128, 2 * DH], F32R, tag="wkcT")
        wqw = const.tile([128, 2 * DH], F32R, tag="wqw")
        wkwT = const.tile([128, NDT * DH], BF16, tag="wkwT")
        for g in range(2):
            nc.gpsimd.dma_start(gt[:, g * T:(g + 1) * T], GT_d.ap()[g * 128:(g + 1) * 128, :])
            nc.gpsimd.dma_start(wqc[:, g * DP:(g + 1) * DP], Wqc_d.ap()[g * 128:(g + 1) * 128, :])
            nc.gpsimd.dma_start(wkcT[:, g * DH:(g + 1) * DH], WkcT_d.ap()[g * 128:(g + 1) * 128, :])
            nc.gpsimd.dma_start(wqw[:, g * DH:(g + 1) * DH], Wqw_d.ap()[g * 128:(g + 1) * 128, :])

        if with_mask:
            mbias = const.tile([1, SPAN], F32R, tag="mbias")
            onesr = const.tile([1, T], F32R, tag="onesr")
            nc.gpsimd.dma_start(mbias[:], mb_d.ap())
            nc.gpsimd.dma_start(onesr[:], ones_d.ap())

        # ---- q^T then qk^T ----
        qT = []
        for p in range(2):
            ps_ = pj.tile([128, 512], F32, tag="pj")
            for g in range(2):
                nc.tensor.matmul(ps_[:, :T], wqc[:, g * DP + p * 128:g * DP + (p + 1) * 128],
                                 gt[:, g * T:(g + 1) * T], start=(g == 0), stop=(g == 1))
            t_ = sb.tile([128, T], F32R, tag=f"qT{p}")
            nc.scalar.activation(t_[:], ps_[:, :T], ActFn.Identity, scale=S_CORE)
            qT.append(t_)
        qkT = []
        for i in range(NDT):
            ps_ = pj.tile([128, 512], F32, tag="pj")
            for p in range(2):
                nc.tensor.matmul(ps_[:, :T], wkcT[:, p * DH + i * 128:p * DH + (i + 1) * 128],
                                 qT[p][:], start=(p == 0), stop=(p == 1))
            t_ = sb.tile([128, T], F32R, tag=f"qkT{i}")
            nc.vector.tensor_copy(t_[:], ps_[:, :T])
            qkT.append(t_)
        if stage == 1:
            dbg = sb.tile([128, NDT * T], F32, tag="dbg")
            for i in range(NDT):
                nc.vector.tensor_copy(dbg[:, i * T:(i + 1) * T], qkT[i][:].bitcast(F32))
            nc.sync.dma_start(zw_d.ap()[:128, :NDT * T], dbg[:])

        # ---- logits chunks + exp (+ per-chunk sums) ----
        hn = []
        if stage >= 2:
            expLs, csums = [], []
            for c in range(NCH):
                ec_ = sexp.tile([T, 256], F32, tag=f"expL{c}")
                cs_ = sexp.tile([T, 1], F32, tag=f"csum{c}")
                expLs.append(ec_)
                csums.append(cs_)
            ht = {}
            for cc in range(SPAN // 512):
                for i in range(NDT):
                    t_ = htp.tile([128, 512], F32R, tag="ht")
                    nc.sync.dma_start(t_[:], HT_d.ap()[i * 128:(i + 1) * 128,
                                                       cc * 512:(cc + 1) * 512])
                    ht[(cc, i)] = t_
                if cc == 0 and stage >= 5:
                    for j in range(NLT):
                        t_ = hpool.tile([128, DH], F32R, tag="hn")
                        nc.scalar.dma_start(t_[:], Hn_d.ap()[j * 128:(j + 1) * 128, :])
                        hn.append(t_)
                    for e in range(NDT):
                        nc.gpsimd.dma_start(wkwT[:, e * DH:(e + 1) * DH],
                                            WkwT_d.ap()[e * 128:(e + 1) * 128, :])
            for cc in range(SPAN // 512):
                ps_ = lg.tile([T, 512], F32, tag="lg")
                for i in range(NDT):
                    nc.tensor.matmul(ps_[:], qkT[i][:], ht[(cc, i)][:],
                                     start=(i == 0), stop=(i == NDT - 1 and not with_mask))
                if with_mask:
                    nc.tensor.matmul(ps_[:], onesr[:], mbias[:, cc * 512:(cc + 1) * 512],
                                     start=False, stop=True)
                for u in range(2):
                    c = 2 * cc + u
                    nc.scalar.activation(expLs[c][:], ps_[:, u * 256:(u + 1) * 256],
                                         ActFn.Exp, accum_out=csums[c][:])
            if stage == 2:
                nc.sync.dma_start(zw_d.ap()[:T, :256], expLs[0][:])

        # ---- transpose expL into [l, t] f32r tiles; denominators ----
        if stage >= 3:
            expLT = []
            for j in range(NLT):
                ps_ = pj.tile([128, 512], F32, tag="pj")
                nc.tensor.transpose(ps_[:, :T], expLs[j // 2][:, (j % 2) * 128:(j % 2) * 128 + 128],
                                    ident[:T, :T])
                t_ = sb.tile([128, T], F32R, tag=f"eT{j}")
                nc.vector.tensor_copy(t_[:], ps_[:, :T])
                expLT.append(t_)
            recs = []
            for j in range(WLOC):
                d0_ = sexp.tile([T, 1], F32, tag=f"d0_{j}")
                d1_ = sexp.tile([T, 1], F32, tag=f"d1_{j}")
                rc_ = sexp.tile([T, 1], F32, tag=f"rc_{j}")
                nc.vector.tensor_add(d0_[:], csums[j][:], csums[j + 1][:])
                nc.vector.tensor_add(d1_[:], csums[j + 2][:], csums[j + 3][:])
                nc.vector.tensor_add(d0_[:], d0_[:], d1_[:])
                nc.vector.reciprocal(rc_[:], d0_[:])
                recs.append(rc_)
            if stage == 3:
                dbg = sb.tile([128, 2 * T], F32, tag="dbg")
                nc.vector.tensor_copy(dbg[:, :T], expLT[0][:].bitcast(F32))
                nc.vector.tensor_copy(dbg[:, T:2 * T], expLT[1][:].bitcast(F32))
                nc.sync.dma_start(zw_d.ap()[:128, :2 * T], dbg[:])
                nc.sync.dma_start(wl_d.ap()[:, :1], recs[0][:])

        # ---- qw -> qw^T(bf16) -> qw2 ----
        if stage >= 4:
            qw = sb.tile([T, DH], F32, tag="qw")
            for h in range(2):
                ps_ = zp.tile([T, 512], F32, tag="zp")
                for g in range(2):
                    nc.tensor.matmul(ps_[:], gt[:, g * T:(g + 1) * T],
                                     wqw[:, g * DH + h * 512:g * DH + (h + 1) * 512],
                                     start=(g == 0), stop=(g == 1))
                nc.scalar.activation(qw[:, h * 512:(h + 1) * 512], ps_[:], ActFn.Identity,
                                     scale=S_WIN)
            qwT = []
            for e in range(NDT):
                ps_ = pj.tile([128, 512], F32, tag="pj")
                nc.tensor.transpose(ps_[:, :T], qw[:, e * 128:(e + 1) * 128], ident[:T, :T])
                t_ = sb.tile([128, T], BF16, tag=f"qwT{e}")
                nc.vector.tensor_copy(t_[:], ps_[:, :T])
                qwT.append(t_)
            qw2 = sb.tile([T, DH], F32, tag="qw2")
            for h in range(2):
                ps_ = zp.tile([T, 512], F32, tag="zp")
                for e in range(NDT):
                    nc.tensor.matmul(ps_[:], qwT[e][:],
                                     wkwT[:, e * DH + h * 512:e * DH + (h + 1) * 512],
                                     start=(e == 0), stop=(e == NDT - 1))
                nc.scalar.activation(qw2[:, h * 512:(h + 1) * 512], ps_[:], ActFn.Identity)
            if stage == 4:
                nc.sync.dma_start(zw_d.ap()[:T, :DH], qw2[:])

        # ---- Zw per window (normalized in PSUM->SBUF copy), wlog inline ----
        if stage >= 5:
            wlog = sexp.tile([T, WLOC], F32, tag="wlog")
            scratch = sexp.tile([T, DH], F32, tag="scratch")
            for j in range(WLOC):
                t_ = sb.tile([T, DH], F32, tag="zw")
                ps_a = zp.tile([T, 512], F32, tag="zp")
                ps_b = zp.tile([T, 512], F32, tag="zp")
                pss = [ps_a, ps_b]
                for k in range(8):
                    for h in range(2):
                        nc.tensor.matmul(pss[h][:], expLT[2 * j + k][:],
                                         hn[2 * j + k][:, h * 512:(h + 1) * 512],
                                         start=(k == 0), stop=(k == 7))
                for h in range(2):
                    nc.vector.tensor_scalar_mul(t_[:, h * 512:(h + 1) * 512], pss[h][:],
                                                recs[j][:])
                nc.sync.dma_start(zw_d.ap()[j * T:(j + 1) * T, :], t_[:])
                if stage >= 7:
                    nc.vector.tensor_mul(scratch[:], t_[:], qw2[:])
                    nc.vector.reduce_sum(wlog[:, j:j + 1], scratch[:],
                                         axis=mybir.AxisListType.X)
            if stage >= 7:
                nc.gpsimd.dma_start(wl_d.ap(), wlog[:])

    nc.compile()
    return nc


def kernel(H, G, Wq_core, Wk_core, Wq_win, Wk_win, attn_mask):
    H = np.asarray(H, dtype=np.float32)
    G = np.asarray(G, dtype=np.float32)
    Wq_core = np.asarray(Wq_core, dtype=np.float32)
    Wk_core = np.asarray(Wk_core, dtype=np.float32)
    Wq_win = np.asarray(Wq_win, dtype=np.float32)
    Wk_win = np.asarray(Wk_win, dtype=np.float32)
    mask = np.asarray(attn_mask).astype(bool)

    with_mask = not bool(mask.all())
    key = ("k", with_mask)
    if key not in _CACHE:
        _CACHE[key] = _build(with_mask)
    nc = _CACHE[key]

    WkcT = np.ascontiguousarray(Wk_core.T)
    WkwT = np.ascontiguousarray(Wk_win.T).astype(ml_dtypes.bfloat16)

    in_maps = []
    for c in range(8):
        b, half = c // 2, c % 2
        lo = 0 if half == 0 else L - SPAN
        im = {
            "Hn": np.ascontiguousarray(H[b, lo:lo + SPAN, :]),
            "HT": np.ascontiguousarray(H[b].T[:, lo:lo + SPAN]),
            "GT": np.ascontiguousarray(G[b].T),
            "Wqc": Wq_core,
            "WkcT": WkcT,
            "Wqw": Wq_win,
            "WkwT": WkwT,
        }
        if with_mask:
            im["maskbias"] = np.where(mask[b, lo:lo + SPAN], 0.0, -1e9).astype(np.float32)[None, :]
            im["onesrow"] = np.ones((1, T), dtype=np.float32)
        in_maps.append(im)

    import os
    prof_dir = os.environ.get("BGW_PROFILE_DIR")
    if prof_dir:
        res = run_bass_kernel_spmd(nc, in_maps, core_ids=list(range(8)),
                                   trace=True, tmpdir=prof_dir)
    else:
        res = run_bass_kernel_spmd(nc, in_maps, core_ids=list(range(8)))
    kernel._last_result = res

    # ---- host combine: tiny cross-window softmax over W=13 ----
    Z = np.empty((B, T, DH), dtype=np.float32)
    for b in range(B):
        zw_full = np.empty((W, T, DH), dtype=np.float32)
        wl_full = np.empty((T, W), dtype=np.float32)
        for half in range(2):
            r = res.results[2 * b + half]
            zw = r["Zw_out"].reshape(WLOC, T, DH)
            wl = r["wlog_out"]
            w0 = 0 if half == 0 else W - WLOC
            zw_full[w0:w0 + WLOC] = zw
            wl_full[:, w0:w0 + WLOC] = wl
        m = wl_full.max(axis=1, keepdims=True)
        e = np.exp(wl_full - m)
        wsm = e / e.sum(axis=1, keepdims=True)          # [T, W]
        Z[b] = np.einsum("tw,wtd->td", wsm, zw_full)
    return Z



# revision 3
# speedup vs baseline: 2.7015x; 2.7015x over previous
"""Trainium2 Bass kernel for nn_BucketedGoWatti (sparse windowed attention pooling).

Math (B=4, L=4096, T=32, DH=1024, DG=256, DP=256, WIN=1024, STRIDE=256, W=13):
  All 13 windows are unions of 4 consecutive 256-wide l-chunks, and the
  per-window logits are slices of one global score matrix
    S[b,t,l] = (qk_b @ H_b^T)[t,l],  qk = (G Wq_core) Wk_core^T * DP^-0.5.
  So the device only computes, per 256-chunk c:
    E = exp(S),  s_c[t] = sum_{l in c} E[t,l],  P_c[t,:] = E[t, c] @ H[c, :]
  and the tiny 13-window combine (Zw = sum4(P)/sum4(s), cross-window softmax
  with qw2 = (G Wq_win) Wk_win^T * DH^-0.5) runs on host in f32.

Sharding: core = 2*b + half owns l in [half*2048, half*2048+2048) of batch b
(8 disjoint 256-chunks; no window overlap duplication). H is shipped in fp16
in both layouts (natural for P, transposed for S); P is returned in fp16.
Numpy-simulated end-to-end rel err ~4e-4 (gate 2e-2).
"""
import numpy as np
from contextlib import ExitStack

import concourse.bacc as bacc
import concourse.tile as tile
import concourse.mybir as mybir
import concourse.masks as masks
from concourse.bass_utils import run_bass_kernel_spmd

F32 = mybir.dt.float32
F16 = mybir.dt.float16
ActFn = mybir.ActivationFunctionType

B, L, T = 4, 4096, 32
DH, DG, DP = 1024, 256, 256
WIN, STRIDE = 1024, 256
W = (L - WIN) // STRIDE + 1          # 13
SPAN = 2048                          # per-core l-span
NLC = SPAN // 512                    # 4 l-chunks of 512
NCH = SPAN // 256                    # 8 chunks of 256 (the window quanta)
S_CORE = 1.0 / float(np.sqrt(DP))
S_WIN = 1.0 / float(np.sqrt(DH))

_CACHE = {}


def _build(with_mask: bool):
    nc = bacc.Bacc("TRN2", debug=False, target_bir_lowering=False)

    # [lc, p, i*512+f] with d = i*128+p, l = lc*512+f  (8KB contiguous/partition)
    HT_d = nc.dram_tensor("HT", [NLC, 128, 4096], F16, kind="ExternalInput")
    # [j, p, s*1024+d] with l = j*512 + s*128 + p     (8KB contiguous/partition)
    Hn_d = nc.dram_tensor("Hn", [NLC, 128, 4096], F16, kind="ExternalInput")
    # [p, i*32+t] = qkT[i*128+p, t]
    QKT_d = nc.dram_tensor("QKT", [128, 256], F16, kind="ExternalInput")
    if with_mask:
        mb_d = nc.dram_tensor("maskbias", [1, SPAN], F16, kind="ExternalInput")
        ones_d = nc.dram_tensor("onesrow", [1, T], F16, kind="ExternalInput")
    # [t, c*1024+d]
    P_d = nc.dram_tensor("P_out", [T, NCH * DH], F16, kind="ExternalOutput")
    S_d = nc.dram_tensor("ssum_out", [T, NCH], F32, kind="ExternalOutput")

    with tile.TileContext(nc) as tc, ExitStack() as ctx:
        const = ctx.enter_context(tc.tile_pool(name="const", bufs=1))
        htp = ctx.enter_context(tc.tile_pool(name="htp", bufs=4))
        hpool = ctx.enter_context(tc.tile_pool(name="hpool", bufs=4))
        ep = ctx.enter_context(tc.tile_pool(name="ep", bufs=2))
        sb = ctx.enter_context(tc.tile_pool(name="sb", bufs=1))
        lg = ctx.enter_context(tc.tile_pool(name="lg", bufs=2, space="PSUM"))
        pj = ctx.enter_context(tc.tile_pool(name="pj", bufs=2, space="PSUM"))
        zp = ctx.enter_context(tc.tile_pool(name="zp", bufs=3, space="PSUM"))

        ident = const.tile([128, 128], F32, tag="ident")
        masks.make_identity(nc, ident[:])

        qkt = const.tile([128, 256], F16, tag="qkt")
        nc.sync.dma_start(qkt[:], QKT_d.ap())
        if with_mask:
            mbias = const.tile([1, SPAN], F16, tag="mbias")
            onesr = const.tile([1, T], F16, tag="onesr")
            nc.gpsimd.dma_start(mbias[:], mb_d.ap())
            nc.gpsimd.dma_start(onesr[:], ones_d.ap())

        # big input stream: strict interleave on one HWDGE ring so the
        # pipeline can start after the first MB
        ht, hn = [], []
        for lc in range(NLC):
            t_ = htp.tile([128, 4096], F16, tag="ht")
            nc.sync.dma_start(t_[:], HT_d.ap()[lc])
            ht.append(t_)
            t2_ = hpool.tile([128, 4096], F16, tag="hn")
            nc.sync.dma_start(t2_[:], Hn_d.ap()[lc])
            hn.append(t2_)

        pout = sb.tile([T, NCH * DH], F16, tag="pout")
        ssum = sb.tile([T, NCH], F32, tag="ssum")

        for lc in range(NLC):
            # S[t, l] for l-chunk lc: contract d via 8 stationary qkT tiles
            sps = lg.tile([T, 512], F32, tag="lg")
            for i in range(8):
                nc.tensor.matmul(sps[:], qkt[:, i * 32:(i + 1) * 32],
                                 ht[lc][:, i * 512:(i + 1) * 512],
                                 start=(i == 0), stop=(i == 7 and not with_mask))
            if with_mask:
                nc.tensor.matmul(sps[:], onesr[:],
                                 mbias[:, lc * 512:(lc + 1) * 512],
                                 start=False, stop=True)
            # E = exp(S) with per-256-chunk row sums
            e_ = ep.tile([T, 512], F32, tag="e")
            for u in range(2):
                c = 2 * lc + u
                nc.scalar.activation(e_[:, u * 256:(u + 1) * 256],
                                     sps[:, u * 256:(u + 1) * 256],
                                     ActFn.Exp, accum_out=ssum[:, c:c + 1])
            # E^T in fp16: 4 PE transposes packed into one PSUM tile, one copy
            pst = pj.tile([128, 128], F32, tag="pj")
            for k in range(4):
                nc.tensor.transpose(pst[:, k * 32:(k + 1) * 32],
                                    e_[:, k * 128:(k + 1) * 128], ident[:T, :T])
            et = sb.tile([128, 128], F16, tag="et")
            nc.vector.tensor_copy(et[:], pst[:])
            # P_c = E^T-weighted sums of Hn rows, 256 l-rows per chunk
            for cc in range(2):
                c = 2 * lc + cc
                for h in range(2):
                    pps = zp.tile([T, 512], F32, tag="zp")
                    for k in range(2):
                        s = 2 * cc + k
                        nc.tensor.matmul(pps[:], et[:, s * 32:(s + 1) * 32],
                                         hn[lc][:, s * 1024 + h * 512:
                                                s * 1024 + h * 512 + 512],
                                         start=(k == 0), stop=(k == 1))
                    off = c * 1024 + h * 512
                    nc.vector.tensor_copy(pout[:, off:off + 512], pps[:])
            if lc == 1:
                nc.scalar.dma_start(P_d.ap()[:, :4096], pout[:, :4096])
        nc.scalar.dma_start(P_d.ap()[:, 4096:], pout[:, 4096:])
        nc.gpsimd.dma_start(S_d.ap(), ssum[:])

    nc.compile()
    return nc


def kernel(H, G, Wq_core, Wk_core, Wq_win, Wk_win, attn_mask):
    H = np.asarray(H, dtype=np.float32)
    G = np.asarray(G, dtype=np.float32)
    Wq_core = np.asarray(Wq_core, dtype=np.float32)
    Wk_core = np.asarray(Wk_core, dtype=np.float32)
    Wq_win = np.asarray(Wq_win, dtype=np.float32)
    Wk_win = np.asarray(Wk_win, dtype=np.float32)
    mask = np.asarray(attn_mask).astype(bool)

    with_mask = not bool(mask.all())
    key = ("k", with_mask)
    if key not in _CACHE:
        _CACHE[key] = _build(with_mask)
    nc = _CACHE[key]

    # host-side tiny G projections (weight-space only, no H involvement)
    qk = (G @ Wq_core) @ Wk_core.T * S_CORE          # [B, T, DH]
    qw2 = (G @ Wq_win) @ Wk_win.T * S_WIN            # [B, T, DH]

    in_maps = []
    for c in range(8):
        b, half = c // 2, c % 2
        l0 = half * SPAN
        H16 = H[b, l0:l0 + SPAN, :].astype(np.float16)          # [2048, 1024]
        HT16 = np.ascontiguousarray(H[b].T[:, l0:l0 + SPAN]).astype(np.float16)
        # [i, p, lc, f] -> [lc, p, i, f]
        HTr = np.ascontiguousarray(
            HT16.reshape(8, 128, NLC, 512).transpose(2, 1, 0, 3)
        ).reshape(NLC, 128, 4096)
        # [j, s, p, f] -> [j, p, s, f]
        Hnr = np.ascontiguousarray(
            H16.reshape(NLC, 4, 128, DH).transpose(0, 2, 1, 3)
        ).reshape(NLC, 128, 4096)
        qkT16 = qk[b].T.astype(np.float16)                      # [1024, 32]
        QKTr = np.ascontiguousarray(
            qkT16.reshape(8, 128, 32).transpose(1, 0, 2)
        ).reshape(128, 256)
        im = {"HT": HTr, "Hn": Hnr, "QKT": QKTr}
        if with_mask:
            im["maskbias"] = np.where(mask[b, l0:l0 + SPAN], np.float16(0.0),
                                      np.float16(-30000.0))[None, :].astype(np.float16)
            im["onesrow"] = np.ones((1, T), dtype=np.float16)
        in_maps.append(im)

    import os
    prof_dir = os.environ.get("BGW_PROFILE_DIR")
    res = None
    if prof_dir:
        try:
            res = run_bass_kernel_spmd(nc, in_maps, core_ids=list(range(8)),
                                       trace=True, tmpdir=prof_dir)
        except Exception:
            res = None
    if res is None:
        res = run_bass_kernel_spmd(nc, in_maps, core_ids=list(range(8)))
    kernel._last_result = res

    # ---- host combine: windows = sums of 4 chunk partials, tiny softmax ----
    NCHB = L // 256                                   # 16 chunks per batch
    Z = np.empty((B, T, DH), dtype=np.float32)
    for b in range(B):
        P = np.empty((NCHB, T, DH), dtype=np.float32)
        ss = np.empty((NCHB, T), dtype=np.float32)
        for half in range(2):
            r = res.results[2 * b + half]
            P[half * NCH:(half + 1) * NCH] = (
                r["P_out"].astype(np.float32).reshape(T, NCH, DH).transpose(1, 0, 2))
            ss[half * NCH:(half + 1) * NCH] = r["ssum_out"].T
        Zw = np.empty((W, T, DH), dtype=np.float32)
        wlog = np.empty((T, W), dtype=np.float32)
        for w in range(W):
            num = P[w] + P[w + 1] + P[w + 2] + P[w + 3]
            den = ss[w] + ss[w + 1] + ss[w + 2] + ss[w + 3]
            Zw[w] = num / den[:, None]
            wlog[:, w] = (Zw[w] * qw2[b]).sum(-1)
        m2 = wlog.max(-1, keepdims=True)
        wsm = np.exp(wlog - m2)
        wsm /= wsm.sum(-1, keepdims=True)
        Z[b] = np.einsum("tw,wtd->td", wsm, Zw)
    return Z


# revision 5
# speedup vs baseline: 2.8024x; 1.0373x over previous
"""Trainium2 Bass kernel for nn_BucketedGoWatti (sparse windowed attention pooling).

Math (B=4, L=4096, T=32, DH=1024, DG=256, DP=256, WIN=1024, STRIDE=256, W=13):
  All 13 windows are unions of 4 consecutive 256-wide l-chunks, and the
  per-window logits are slices of one global score matrix
    S[b,t,l] = (qk_b @ H_b^T)[t,l],  qk = (G Wq_core) Wk_core^T * DP^-0.5.
  So the device only computes, per 256-chunk c:
    E = exp(S),  s_c[t] = sum_{l in c} E[t,l],  P_c[t,:] = E[t, c] @ H[c, :]
  and the tiny 13-window combine (Zw = sum4(P)/sum4(s), cross-window softmax
  with qw2 = (G Wq_win) Wk_win^T * DH^-0.5) runs on host in f32.

Sharding: core = 2*b + half owns l in [half*2048, half*2048+2048) of batch b
(8 disjoint 256-chunks; no window overlap duplication). H is shipped in fp16
in both layouts (natural for P, transposed for S); P is returned in fp16.
Numpy-simulated end-to-end rel err ~4e-4 (gate 2e-2).
"""
import numpy as np
from contextlib import ExitStack

import concourse.bacc as bacc
import concourse.tile as tile
import concourse.mybir as mybir
import concourse.masks as masks
from concourse.bass_utils import run_bass_kernel_spmd

F32 = mybir.dt.float32
F16 = mybir.dt.float16
ActFn = mybir.ActivationFunctionType

B, L, T = 4, 4096, 32
DH, DG, DP = 1024, 256, 256
WIN, STRIDE = 1024, 256
W = (L - WIN) // STRIDE + 1          # 13
SPAN = 2048                          # per-core l-span
NLC = SPAN // 512                    # 4 l-chunks of 512
NCH = SPAN // 256                    # 8 chunks of 256 (the window quanta)
S_CORE = 1.0 / float(np.sqrt(DP))
S_WIN = 1.0 / float(np.sqrt(DH))

_CACHE = {}


def _build(with_mask: bool):
    nc = bacc.Bacc("TRN2", debug=False, target_bir_lowering=False)

    # [lc, p, i*512+f] with d = i*128+p, l = lc*512+f  (8KB contiguous/partition)
    HT_d = nc.dram_tensor("HT", [NLC, 128, 4096], F16, kind="ExternalInput")
    # [j, p, s*1024+d] with l = j*512 + s*128 + p     (8KB contiguous/partition)
    Hn_d = nc.dram_tensor("Hn", [NLC, 128, 4096], F16, kind="ExternalInput")
    # [p, i*32+t] = qkT[i*128+p, t]
    QKT_d = nc.dram_tensor("QKT", [128, 256], F16, kind="ExternalInput")
    if with_mask:
        mb_d = nc.dram_tensor("maskbias", [1, SPAN], F16, kind="ExternalInput")
        ones_d = nc.dram_tensor("onesrow", [1, T], F16, kind="ExternalInput")
    # [t, c*1024+d]
    P_d = nc.dram_tensor("P_out", [T, NCH * DH], F16, kind="ExternalOutput")
    S_d = nc.dram_tensor("ssum_out", [T, NCH], F32, kind="ExternalOutput")

    with tile.TileContext(nc) as tc, ExitStack() as ctx:
        const = ctx.enter_context(tc.tile_pool(name="const", bufs=1))
        htp = ctx.enter_context(tc.tile_pool(name="htp", bufs=4))
        hpool = ctx.enter_context(tc.tile_pool(name="hpool", bufs=4))
        ep = ctx.enter_context(tc.tile_pool(name="ep", bufs=2))
        sb = ctx.enter_context(tc.tile_pool(name="sb", bufs=1))
        lg = ctx.enter_context(tc.tile_pool(name="lg", bufs=2, space="PSUM"))
        pj = ctx.enter_context(tc.tile_pool(name="pj", bufs=2, space="PSUM"))
        zp = ctx.enter_context(tc.tile_pool(name="zp", bufs=3, space="PSUM"))
        wp = ctx.enter_context(tc.tile_pool(name="wp", bufs=1, space="PSUM"))

        ident = const.tile([128, 128], F32, tag="ident")
        masks.make_identity(nc, ident[:])

        # ~4us of dummy f32 matmuls while the first DMAs stream in: trips the
        # PE HAM un-throttle (needs ~3.4us sustained busy) so the real matmuls
        # run at 2.4GHz instead of the cold 1.2GHz default.
        warm = wp.tile([128, 128], F32, tag="warm")
        for _ in range(9):
            nc.tensor.matmul(warm[:], ident[:], ident[:], start=True, stop=True)

        qkt = const.tile([128, 256], F16, tag="qkt")
        nc.sync.dma_start(qkt[:], QKT_d.ap())
        if with_mask:
            mbias = const.tile([1, SPAN], F16, tag="mbias")
            onesr = const.tile([1, T], F16, tag="onesr")
            nc.gpsimd.dma_start(mbias[:], mb_d.ap())
            nc.gpsimd.dma_start(onesr[:], ones_d.ap())

        # big input stream: strict interleave on one HWDGE ring so the
        # pipeline can start after the first MB
        ht, hn = [], []
        for lc in range(NLC):
            t_ = htp.tile([128, 4096], F16, tag="ht")
            nc.sync.dma_start(t_[:], HT_d.ap()[lc])
            ht.append(t_)
            t2_ = hpool.tile([128, 4096], F16, tag="hn")
            nc.sync.dma_start(t2_[:], Hn_d.ap()[lc])
            hn.append(t2_)

        pout = sb.tile([T, NCH * DH], F16, tag="pout")
        ssum = sb.tile([T, NCH], F32, tag="ssum")

        for lc in range(NLC):
            # S[t, l] for l-chunk lc: contract d via 8 stationary qkT tiles
            sps = lg.tile([T, 512], F32, tag="lg")
            for i in range(8):
                nc.tensor.matmul(sps[:], qkt[:, i * 32:(i + 1) * 32],
                                 ht[lc][:, i * 512:(i + 1) * 512],
                                 start=(i == 0), stop=(i == 7 and not with_mask))
            if with_mask:
                nc.tensor.matmul(sps[:], onesr[:],
                                 mbias[:, lc * 512:(lc + 1) * 512],
                                 start=False, stop=True)
            # E = exp(S) with per-256-chunk row sums
            e_ = ep.tile([T, 512], F32, tag="e")
            for u in range(2):
                c = 2 * lc + u
                nc.scalar.activation(e_[:, u * 256:(u + 1) * 256],
                                     sps[:, u * 256:(u + 1) * 256],
                                     ActFn.Exp, accum_out=ssum[:, c:c + 1])
            # E^T in fp16: 4 PE transposes packed into one PSUM tile, one copy
            pst = pj.tile([128, 128], F32, tag="pj")
            for k in range(4):
                nc.tensor.transpose(pst[:, k * 32:(k + 1) * 32],
                                    e_[:, k * 128:(k + 1) * 128], ident[:T, :T])
            et = sb.tile([128, 128], F16, tag="et")
            nc.vector.tensor_copy(et[:], pst[:])
            # P_c = E^T-weighted sums of Hn rows, 256 l-rows per chunk
            for cc in range(2):
                c = 2 * lc + cc
                for h in range(2):
                    pps = zp.tile([T, 512], F32, tag="zp")
                    for k in range(2):
                        s = 2 * cc + k
                        nc.tensor.matmul(pps[:], et[:, s * 32:(s + 1) * 32],
                                         hn[lc][:, s * 1024 + h * 512:
                                                s * 1024 + h * 512 + 512],
                                         start=(k == 0), stop=(k == 1))
                    off = c * 1024 + h * 512
                    if h == 0:
                        nc.vector.tensor_copy(pout[:, off:off + 512], pps[:])
                    else:
                        nc.scalar.activation(pout[:, off:off + 512], pps[:],
                                             ActFn.Identity)
            # ship this l-chunk's quarter of P as soon as its copies land
            nc.scalar.dma_start(P_d.ap()[:, lc * 2048:(lc + 1) * 2048],
                                pout[:, lc * 2048:(lc + 1) * 2048])
        nc.scalar.dma_start(S_d.ap(), ssum[:])

    nc.compile()
    return nc


def kernel(H, G, Wq_core, Wk_core, Wq_win, Wk_win, attn_mask):
    H = np.asarray(H, dtype=np.float32)
    G = np.asarray(G, dtype=np.float32)
    Wq_core = np.asarray(Wq_core, dtype=np.float32)
    Wk_core = np.asarray(Wk_core, dtype=np.float32)
    Wq_win = np.asarray(Wq_win, dtype=np.float32)
    Wk_win = np.asarray(Wk_win, dtype=np.float32)
    mask = np.asarray(attn_mask).astype(bool)

    with_mask = not bool(mask.all())
    key = ("k", with_mask)
    if key not in _CACHE:
        _CACHE[key] = _build(with_mask)
    nc = _CACHE[key]

    # host-side tiny G projections (weight-space only, no H involvement)
    qk = (G @ Wq_core) @ Wk_core.T * S_CORE          # [B, T, DH]
    qw2 = (G @ Wq_win) @ Wk_win.T * S_WIN            # [B, T, DH]

    in_maps = []
    for c in range(8):
        b, half = c // 2, c % 2
        l0 = half * SPAN
        H16 = H[b, l0:l0 + SPAN, :].astype(np.float16)          # [2048, 1024]
        HT16 = np.ascontiguousarray(H[b].T[:, l0:l0 + SPAN]).astype(np.float16)
        # [i, p, lc, f] -> [lc, p, i, f]
        HTr = np.ascontiguousarray(
            HT16.reshape(8, 128, NLC, 512).transpose(2, 1, 0, 3)
        ).reshape(NLC, 128, 4096)
        # [j, s, p, f] -> [j, p, s, f]
        Hnr = np.ascontiguousarray(
            H16.reshape(NLC, 4, 128, DH).transpose(0, 2, 1, 3)
        ).reshape(NLC, 128, 4096)
        qkT16 = qk[b].T.astype(np.float16)                      # [1024, 32]
        QKTr = np.ascontiguousarray(
            qkT16.reshape(8, 128, 32).transpose(1, 0, 2)
        ).reshape(128, 256)
        im = {"HT": HTr, "Hn": Hnr, "QKT": QKTr}
        if with_mask:
            im["maskbias"] = np.where(mask[b, l0:l0 + SPAN], np.float16(0.0),
                                      np.float16(-30000.0))[None, :].astype(np.float16)
            im["onesrow"] = np.ones((1, T), dtype=np.float16)
        in_maps.append(im)

    import os
    prof_dir = os.environ.get("BGW_PROFILE_DIR")
    res = None
    if prof_dir:
        try:
            res = run_bass_kernel_spmd(nc, in_maps, core_ids=list(range(8)),
                                       trace=True, tmpdir=prof_dir)
        except Exception:
            res = None
    if res is None:
        res = run_bass_kernel_spmd(nc, in_maps, core_ids=list(range(8)))
    kernel._last_result = res

    # ---- host combine: windows = sums of 4 chunk partials, tiny softmax ----
    NCHB = L // 256                                   # 16 chunks per batch
    Z = np.empty((B, T, DH), dtype=np.float32)
    for b in range(B):
        P = np.empty((NCHB, T, DH), dtype=np.float32)
        ss = np.empty((NCHB, T), dtype=np.float32)
        for half in range(2):
            r = res.results[2 * b + half]
            P[half * NCH:(half + 1) * NCH] = (
                r["P_out"].astype(np.float32).reshape(T, NCH, DH).transpose(1, 0, 2))
            ss[half * NCH:(half + 1) * NCH] = r["ssum_out"].T
        Zw = np.empty((W, T, DH), dtype=np.float32)
        wlog = np.empty((T, W), dtype=np.float32)
        for w in range(W):
            num = P[w] + P[w + 1] + P[w + 2] + P[w + 3]
            den = ss[w] + ss[w + 1] + ss[w + 2] + ss[w + 3]
            Zw[w] = num / den[:, None]
            wlog[:, w] = (Zw[w] * qw2[b]).sum(-1)
        m2 = wlog.max(-1, keepdims=True)
        wsm = np.exp(wlog - m2)
        wsm /= wsm.sum(-1, keepdims=True)
        Z[b] = np.einsum("tw,wtd->td", wsm, Zw)
    return Z


# revision 6
# speedup vs baseline: 3.0421x; 1.0855x over previous
"""Trainium2 Bass kernel for nn_BucketedGoWatti (sparse windowed attention pooling).

Math (B=4, L=4096, T=32, DH=1024, DG=256, DP=256, WIN=1024, STRIDE=256, W=13):
  All 13 windows are unions of 4 consecutive 256-wide l-chunks, and the
  per-window logits are slices of one global score matrix
    S[b,t,l] = (qk_b @ H_b^T)[t,l],  qk = (G Wq_core) Wk_core^T * DP^-0.5.
  So the device only computes, per 256-chunk c:
    E = exp(S),  s_c[t] = sum_{l in c} E[t,l],  P_c[t,:] = E[t, c] @ H[c, :]
  and the tiny 13-window combine (Zw = sum4(P)/sum4(s), cross-window softmax
  with qw2 = (G Wq_win) Wk_win^T * DH^-0.5) runs on host in f32.

Sharding: core = 2*b + half owns l in [half*2048, half*2048+2048) of batch b
(8 disjoint 256-chunks; no window overlap duplication). H is shipped in fp16
in both layouts (natural for P, transposed for S); P is returned in fp16.

PE shape: T=32 output rows -> 4x column tiling (tile_position col groups),
so S and P stream 4 concurrent matmuls; the cross-group S reduce is one f32r
matmul against a [128,32] stacked-identity. Numpy-sim rel err ~4e-4 (gate 2e-2).
"""
import numpy as np
from contextlib import ExitStack

import concourse.bacc as bacc
import concourse.tile as tile
import concourse.mybir as mybir
import concourse.masks as masks
from concourse.bass_utils import run_bass_kernel_spmd

F32 = mybir.dt.float32
F32R = mybir.dt.float32r
F16 = mybir.dt.float16
ActFn = mybir.ActivationFunctionType

B, L, T = 4, 4096, 32
DH, DG, DP = 1024, 256, 256
WIN, STRIDE = 1024, 256
W = (L - WIN) // STRIDE + 1          # 13
SPAN = 2048                          # per-core l-span
NLC = SPAN // 512                    # 4 l-chunks of 512
NCH = SPAN // 256                    # 8 chunks of 256 (the window quanta)
S_CORE = 1.0 / float(np.sqrt(DP))
S_WIN = 1.0 / float(np.sqrt(DH))

_CACHE = {}


def _build(with_mask: bool):
    nc = bacc.Bacc("TRN2", debug=False, target_bir_lowering=False)

    # [lc, p, i*512+f] with d = i*128+p, l = lc*512+f  (8KB contiguous/partition)
    HT_d = nc.dram_tensor("HT", [NLC, 128, 4096], F16, kind="ExternalInput")
    # [j, p, s*1024+d] with l = j*512 + s*128 + p     (8KB contiguous/partition)
    Hn_d = nc.dram_tensor("Hn", [NLC, 128, 4096], F16, kind="ExternalInput")
    # [p, i*32+t] = qkT[i*128+p, t]
    QKT_d = nc.dram_tensor("QKT", [128, 256], F16, kind="ExternalInput")
    # stacked identity for the cross-col-group reduce
    SEL_d = nc.dram_tensor("SEL", [128, 32], F32R, kind="ExternalInput")
    if with_mask:
        mb_d = nc.dram_tensor("maskbias", [1, SPAN], F32R, kind="ExternalInput")
        ones_d = nc.dram_tensor("onesrow", [1, T], F32R, kind="ExternalInput")
    # row 32*(2*cc+h)+t, col lc*512+d'  ->  P[2lc+cc][t, h*512+d']
    P_d = nc.dram_tensor("P_out", [128, NLC * 512], F16, kind="ExternalOutput")
    S_d = nc.dram_tensor("ssum_out", [T, NCH], F32, kind="ExternalOutput")

    with tile.TileContext(nc) as tc, ExitStack() as ctx:
        const = ctx.enter_context(tc.tile_pool(name="const", bufs=1))
        htp = ctx.enter_context(tc.tile_pool(name="htp", bufs=4))
        hpool = ctx.enter_context(tc.tile_pool(name="hpool", bufs=4))
        ep = ctx.enter_context(tc.tile_pool(name="ep", bufs=2))
        spl = ctx.enter_context(tc.tile_pool(name="spl", bufs=2))
        etp = ctx.enter_context(tc.tile_pool(name="etp", bufs=2))
        sb = ctx.enter_context(tc.tile_pool(name="sb", bufs=1))
        sp4 = ctx.enter_context(tc.tile_pool(name="sp4", bufs=2, space="PSUM"))
        lg = ctx.enter_context(tc.tile_pool(name="lg", bufs=2, space="PSUM"))
        pj = ctx.enter_context(tc.tile_pool(name="pj", bufs=2, space="PSUM"))
        zp = ctx.enter_context(tc.tile_pool(name="zp", bufs=2, space="PSUM"))

        ident = const.tile([128, 128], F32, tag="ident")
        masks.make_identity(nc, ident[:])
        identh = const.tile([32, 32], F16, tag="identh")
        nc.vector.tensor_copy(identh[:], ident[:32, :32])

        sel = const.tile([128, 32], F32R, tag="sel")
        nc.sync.dma_start(sel[:], SEL_d.ap())
        qkt = const.tile([128, 256], F16, tag="qkt")
        nc.sync.dma_start(qkt[:], QKT_d.ap())
        if with_mask:
            mbias = const.tile([1, SPAN], F32R, tag="mbias")
            onesr = const.tile([1, T], F32R, tag="onesr")
            nc.gpsimd.dma_start(mbias[:], mb_d.ap())
            nc.gpsimd.dma_start(onesr[:], ones_d.ap())

        # big input stream: strict interleave on one HWDGE ring so the
        # pipeline can start after the first MB
        ht, hn = [], []
        for lc in range(NLC):
            t_ = htp.tile([128, 4096], F16, tag="ht")
            nc.sync.dma_start(t_[:], HT_d.ap()[lc])
            ht.append(t_)
            t2_ = hpool.tile([128, 4096], F16, tag="hn")
            nc.sync.dma_start(t2_[:], Hn_d.ap()[lc])
            hn.append(t2_)

        # ~4us of dummy f32 matmuls while the first DMAs stream in: trips the
        # PE HAM un-throttle (~3.4us sustained busy) so later matmuls have a
        # chance to run at 2.4GHz instead of the cold 1.2GHz default.
        warm = sp4.tile([128, 512], F32, tag="spart")
        for _ in range(9):
            nc.tensor.matmul(warm[:, :128], ident[:], ident[:],
                             start=True, stop=True)

        pout = sb.tile([128, NLC * 512], F16, tag="pout")
        ssum = sb.tile([T, NCH], F32, tag="ssum")

        for lc in range(NLC):
            # --- S[t, l] for this l-chunk: 4 col-groups x 2 d-tiles each ---
            spart = sp4.tile([128, 512], F32, tag="spart")
            for k in range(2):
                for g in range(4):
                    i = g + 4 * k
                    nc.tensor.matmul(spart[32 * g:32 * g + 32, :],
                                     qkt[:, i * 32:(i + 1) * 32],
                                     ht[lc][:, i * 512:(i + 1) * 512],
                                     start=(k == 0), stop=(k == 1),
                                     tile_position=(0, 32 * g))
            sparts = spl.tile([128, 512], F32R, tag="sps")
            nc.vector.tensor_copy(sparts[:], spart[:])
            sps = lg.tile([T, 512], F32, tag="lg")
            nc.tensor.matmul(sps[:], sel[:], sparts[:],
                             start=True, stop=(not with_mask))
            if with_mask:
                nc.tensor.matmul(sps[:], onesr[:],
                                 mbias[:, lc * 512:(lc + 1) * 512],
                                 start=False, stop=True)
            # --- E = exp(S) in fp16, with per-256-chunk row sums ---
            e_ = ep.tile([T, 512], F16, tag="e")
            for u in range(2):
                c = 2 * lc + u
                nc.scalar.activation(e_[:, u * 256:(u + 1) * 256],
                                     sps[:, u * 256:(u + 1) * 256],
                                     ActFn.Exp, accum_out=ssum[:, c:c + 1])
            # --- E^T fp16: 4 PE transposes packed in one PSUM tile, 1 copy ---
            pst = pj.tile([128, 128], F16, tag="pj")
            for k in range(4):
                nc.tensor.transpose(pst[:, k * 32:(k + 1) * 32],
                                    e_[:, k * 128:(k + 1) * 128], identh[:])
            et = etp.tile([128, 128], F16, tag="et")
            nc.vector.tensor_copy(et[:], pst[:])
            # --- P: 4 col-groups j=(cc,h), 2 l-subtiles each ---
            ppack = zp.tile([128, 512], F32, tag="zp")
            for k in range(2):
                for j in range(4):
                    cc, h = j // 2, j % 2
                    s = 2 * cc + k
                    nc.tensor.matmul(ppack[32 * j:32 * j + 32, :],
                                     et[:, s * 32:(s + 1) * 32],
                                     hn[lc][:, s * 1024 + h * 512:
                                            s * 1024 + h * 512 + 512],
                                     start=(k == 0), stop=(k == 1),
                                     tile_position=(0, 32 * j))
            if lc % 2 == 0:
                nc.vector.tensor_copy(pout[:, lc * 512:(lc + 1) * 512], ppack[:])
            else:
                nc.scalar.activation(pout[:, lc * 512:(lc + 1) * 512], ppack[:],
                                     ActFn.Identity)
            # ship this l-chunk's quarter of P as soon as its copy lands
            nc.scalar.dma_start(P_d.ap()[:, lc * 512:(lc + 1) * 512],
                                pout[:, lc * 512:(lc + 1) * 512])
        nc.scalar.dma_start(S_d.ap(), ssum[:])

    nc.compile()
    return nc


def kernel(H, G, Wq_core, Wk_core, Wq_win, Wk_win, attn_mask):
    H = np.asarray(H, dtype=np.float32)
    G = np.asarray(G, dtype=np.float32)
    Wq_core = np.asarray(Wq_core, dtype=np.float32)
    Wk_core = np.asarray(Wk_core, dtype=np.float32)
    Wq_win = np.asarray(Wq_win, dtype=np.float32)
    Wk_win = np.asarray(Wk_win, dtype=np.float32)
    mask = np.asarray(attn_mask).astype(bool)

    with_mask = not bool(mask.all())
    key = ("k", with_mask)
    if key not in _CACHE:
        _CACHE[key] = _build(with_mask)
    nc = _CACHE[key]

    # host-side tiny G projections (weight-space only, no H involvement)
    qk = (G @ Wq_core) @ Wk_core.T * S_CORE          # [B, T, DH]
    qw2 = (G @ Wq_win) @ Wk_win.T * S_WIN            # [B, T, DH]
    selmat = np.tile(np.eye(32, dtype=np.float32), (4, 1))  # [128, 32]

    in_maps = []
    for c in range(8):
        b, half = c // 2, c % 2
        l0 = half * SPAN
        H16 = H[b, l0:l0 + SPAN, :].astype(np.float16)          # [2048, 1024]
        HT16 = np.ascontiguousarray(H[b].T[:, l0:l0 + SPAN]).astype(np.float16)
        # [i, p, lc, f] -> [lc, p, i, f]
        HTr = np.ascontiguousarray(
            HT16.reshape(8, 128, NLC, 512).transpose(2, 1, 0, 3)
        ).reshape(NLC, 128, 4096)
        # [j, s, p, f] -> [j, p, s, f]
        Hnr = np.ascontiguousarray(
            H16.reshape(NLC, 4, 128, DH).transpose(0, 2, 1, 3)
        ).reshape(NLC, 128, 4096)
        qkT16 = qk[b].T.astype(np.float16)                      # [1024, 32]
        QKTr = np.ascontiguousarray(
            qkT16.reshape(8, 128, 32).transpose(1, 0, 2)
        ).reshape(128, 256)
        im = {"HT": HTr, "Hn": Hnr, "QKT": QKTr, "SEL": selmat}
        if with_mask:
            im["maskbias"] = np.where(mask[b, l0:l0 + SPAN], 0.0,
                                      -1e9).astype(np.float32)[None, :]
            im["onesrow"] = np.ones((1, T), dtype=np.float32)
        in_maps.append(im)

    import os
    prof_dir = os.environ.get("BGW_PROFILE_DIR")
    res = None
    if prof_dir:
        try:
            res = run_bass_kernel_spmd(nc, in_maps, core_ids=list(range(8)),
                                       trace=True, tmpdir=prof_dir)
        except Exception:
            res = None
    if res is None:
        res = run_bass_kernel_spmd(nc, in_maps, core_ids=list(range(8)))
    kernel._last_result = res

    # ---- host combine: windows = sums of 4 chunk partials, tiny softmax ----
    NCHB = L // 256                                   # 16 chunks per batch
    Z = np.empty((B, T, DH), dtype=np.float32)
    for b in range(B):
        P = np.empty((NCHB, T, DH), dtype=np.float32)
        ss = np.empty((NCHB, T), dtype=np.float32)
        for half in range(2):
            r = res.results[2 * b + half]
            arr = r["P_out"].astype(np.float32).reshape(4, 32, NLC, 512)
            for lc in range(NLC):
                for cc in range(2):
                    for h in range(2):
                        P[half * NCH + 2 * lc + cc, :, h * 512:(h + 1) * 512] = \
                            arr[2 * cc + h, :, lc, :]
            ss[half * NCH:(half + 1) * NCH] = r["ssum_out"].T
        Zw = np.empty((W, T, DH), dtype=np.float32)
        wlog = np.empty((T, W), dtype=np.float32)
        for w in range(W):
            num = P[w] + P[w + 1] + P[w + 2] + P[w + 3]
            den = ss[w] + ss[w + 1] + ss[w + 2] + ss[w + 3]
            Zw[w] = num / den[:, None]
            wlog[:, w] = (Zw[w] * qw2[b]).sum(-1)
        m2 = wlog.max(-1, keepdims=True)
        wsm = np.exp(wlog - m2)
        wsm /= wsm.sum(-1, keepdims=True)
        Z[b] = np.einsum("tw,wtd->td", wsm, Zw)
    return Z


# revision 7
# speedup vs baseline: 3.0906x; 1.0159x over previous
"""Trainium2 Bass kernel for nn_BucketedGoWatti (sparse windowed attention pooling).

Math (B=4, L=4096, T=32, DH=1024, DG=256, DP=256, WIN=1024, STRIDE=256, W=13):
  All 13 windows are unions of 4 consecutive 256-wide l-chunks, and the
  per-window logits are slices of one global score matrix
    S[b,t,l] = (qk_b @ H_b^T)[t,l],  qk = (G Wq_core) Wk_core^T * DP^-0.5.
  So the device only computes, per 256-chunk c:
    E = exp(S),  s_c[t] = sum_{l in c} E[t,l],  P_c[t,:] = E[t, c] @ H[c, :]
  and the tiny 13-window combine (Zw = sum4(P)/sum4(s), cross-window softmax
  with qw2 = (G Wq_win) Wk_win^T * DH^-0.5) runs on host in f32.

Sharding: core = 2*b + half owns l in [half*2048, half*2048+2048) of batch b
(8 disjoint 256-chunks; no window overlap duplication). H is shipped in fp16
in both layouts (natural for P, transposed for S); P is returned in fp16.

PE shape: T=32 output rows -> 4x column tiling (tile_position col groups),
so S and P stream 4 concurrent matmuls; the cross-group S reduce is one f32r
matmul against a [128,32] stacked-identity. Numpy-sim rel err ~4e-4 (gate 2e-2).
"""
import numpy as np
from contextlib import ExitStack

import concourse.bacc as bacc
import concourse.tile as tile
import concourse.mybir as mybir
import concourse.masks as masks
from concourse.bass_utils import run_bass_kernel_spmd

F32 = mybir.dt.float32
F32R = mybir.dt.float32r
F16 = mybir.dt.float16
ActFn = mybir.ActivationFunctionType

B, L, T = 4, 4096, 32
DH, DG, DP = 1024, 256, 256
WIN, STRIDE = 1024, 256
W = (L - WIN) // STRIDE + 1          # 13
SPAN = 2048                          # per-core l-span
NLC = SPAN // 512                    # 4 l-chunks of 512
NCH = SPAN // 256                    # 8 chunks of 256 (the window quanta)
S_CORE = 1.0 / float(np.sqrt(DP))
S_WIN = 1.0 / float(np.sqrt(DH))

_CACHE = {}


def _build(with_mask: bool):
    nc = bacc.Bacc("TRN2", debug=False, target_bir_lowering=False)

    # [lc, p, i*512+f] with d = i*128+p, l = lc*512+f  (8KB contiguous/partition)
    HT_d = nc.dram_tensor("HT", [NLC, 128, 4096], F16, kind="ExternalInput")
    # [j, p, s*1024+d] with l = j*512 + s*128 + p     (8KB contiguous/partition)
    Hn_d = nc.dram_tensor("Hn", [NLC, 128, 4096], F16, kind="ExternalInput")
    # [p, i*32+t] = qkT[i*128+p, t]
    QKT_d = nc.dram_tensor("QKT", [128, 256], F16, kind="ExternalInput")
    # stacked identity for the cross-col-group reduce
    SEL_d = nc.dram_tensor("SEL", [128, 32], F32R, kind="ExternalInput")
    if with_mask:
        mb_d = nc.dram_tensor("maskbias", [1, SPAN], F32R, kind="ExternalInput")
        ones_d = nc.dram_tensor("onesrow", [1, T], F32R, kind="ExternalInput")
    # row 32*(2*cc+h)+t, col lc*512+d'  ->  P[2lc+cc][t, h*512+d']
    P_d = nc.dram_tensor("P_out", [128, NLC * 512], F16, kind="ExternalOutput")
    S_d = nc.dram_tensor("ssum_out", [T, NCH], F32, kind="ExternalOutput")

    with tile.TileContext(nc) as tc, ExitStack() as ctx:
        const = ctx.enter_context(tc.tile_pool(name="const", bufs=1))
        htp = ctx.enter_context(tc.tile_pool(name="htp", bufs=4))
        hpool = ctx.enter_context(tc.tile_pool(name="hpool", bufs=4))
        ep = ctx.enter_context(tc.tile_pool(name="ep", bufs=2))
        spl = ctx.enter_context(tc.tile_pool(name="spl", bufs=2))
        etp = ctx.enter_context(tc.tile_pool(name="etp", bufs=2))
        sb = ctx.enter_context(tc.tile_pool(name="sb", bufs=1))
        sp4 = ctx.enter_context(tc.tile_pool(name="sp4", bufs=2, space="PSUM"))
        lg = ctx.enter_context(tc.tile_pool(name="lg", bufs=2, space="PSUM"))
        pj = ctx.enter_context(tc.tile_pool(name="pj", bufs=2, space="PSUM"))
        zp = ctx.enter_context(tc.tile_pool(name="zp", bufs=2, space="PSUM"))

        ident = const.tile([128, 128], F32, tag="ident")
        masks.make_identity(nc, ident[:])
        identh = const.tile([32, 32], F16, tag="identh")
        nc.vector.tensor_copy(identh[:], ident[:32, :32])

        sel = const.tile([128, 32], F32R, tag="sel")
        nc.sync.dma_start(sel[:], SEL_d.ap())
        qkt = const.tile([128, 256], F16, tag="qkt")
        nc.sync.dma_start(qkt[:], QKT_d.ap())
        if with_mask:
            mbias = const.tile([1, SPAN], F32R, tag="mbias")
            onesr = const.tile([1, T], F32R, tag="onesr")
            nc.gpsimd.dma_start(mbias[:], mb_d.ap())
            nc.gpsimd.dma_start(onesr[:], ones_d.ap())

        # big input stream: strict interleave on one HWDGE ring so the
        # pipeline can start after the first MB
        ht, hn = [], []
        for lc in range(NLC):
            t_ = htp.tile([128, 4096], F16, tag="ht")
            nc.sync.dma_start(t_[:], HT_d.ap()[lc])
            ht.append(t_)
            t2_ = hpool.tile([128, 4096], F16, tag="hn")
            nc.sync.dma_start(t2_[:], Hn_d.ap()[lc])
            hn.append(t2_)

        # ~4us of dummy f32 matmuls while the first DMAs stream in: trips the
        # PE HAM un-throttle (~3.4us sustained busy) so later matmuls have a
        # chance to run at 2.4GHz instead of the cold 1.2GHz default.
        warm = sp4.tile([128, 512], F32, tag="spart")
        for _ in range(9):
            nc.tensor.matmul(warm[:, :128], ident[:], ident[:],
                             start=True, stop=True)

        pout = sb.tile([128, NLC * 512], F16, tag="pout")
        ssum = sb.tile([T, NCH], F32, tag="ssum")
        es, ets = {}, {}

        def s_phase(lc):
            # --- S[t, l] for this l-chunk: 4 col-groups x 2 d-tiles each ---
            spart = sp4.tile([128, 512], F32, tag="spart")
            for k in range(2):
                for g in range(4):
                    i = g + 4 * k
                    nc.tensor.matmul(spart[32 * g:32 * g + 32, :],
                                     qkt[:, i * 32:(i + 1) * 32],
                                     ht[lc][:, i * 512:(i + 1) * 512],
                                     start=(k == 0), stop=(k == 1),
                                     tile_position=(0, 32 * g))
            sparts = spl.tile([128, 512], F32R, tag="sps")
            nc.vector.tensor_copy(sparts[:], spart[:])
            sps = lg.tile([T, 512], F32, tag="lg")
            nc.tensor.matmul(sps[:], sel[:], sparts[:],
                             start=True, stop=(not with_mask))
            if with_mask:
                nc.tensor.matmul(sps[:], onesr[:],
                                 mbias[:, lc * 512:(lc + 1) * 512],
                                 start=False, stop=True)
            # --- E = exp(S) in fp16, with per-256-chunk row sums ---
            e_ = ep.tile([T, 512], F16, tag="e")
            for u in range(2):
                c = 2 * lc + u
                nc.scalar.activation(e_[:, u * 256:(u + 1) * 256],
                                     sps[:, u * 256:(u + 1) * 256],
                                     ActFn.Exp, accum_out=ssum[:, c:c + 1])
            es[lc] = e_

        def tp_phase(lc):
            # --- E^T fp16: 4 PE transposes packed in one PSUM tile, 1 copy ---
            e_ = es[lc]
            pst = pj.tile([128, 128], F16, tag="pj")
            for k in range(4):
                nc.tensor.transpose(pst[:, k * 32:(k + 1) * 32],
                                    e_[:, k * 128:(k + 1) * 128], identh[:])
            et = etp.tile([128, 128], F16, tag="et")
            nc.vector.tensor_copy(et[:], pst[:])
            # --- P: 4 col-groups j=(cc,h), 2 l-subtiles each ---
            ppack = zp.tile([128, 512], F32, tag="zp")
            for k in range(2):
                for j in range(4):
                    cc, h = j // 2, j % 2
                    s = 2 * cc + k
                    nc.tensor.matmul(ppack[32 * j:32 * j + 32, :],
                                     et[:, s * 32:(s + 1) * 32],
                                     hn[lc][:, s * 1024 + h * 512:
                                            s * 1024 + h * 512 + 512],
                                     start=(k == 0), stop=(k == 1),
                                     tile_position=(0, 32 * j))
            if lc % 2 == 0:
                nc.vector.tensor_copy(pout[:, lc * 512:(lc + 1) * 512], ppack[:])
            else:
                nc.scalar.activation(pout[:, lc * 512:(lc + 1) * 512], ppack[:],
                                     ActFn.Identity)
            # ship this l-chunk's quarter of P as soon as its copy lands
            nc.scalar.dma_start(P_d.ap()[:, lc * 512:(lc + 1) * 512],
                                pout[:, lc * 512:(lc + 1) * 512])

        # software pipeline with a one-chunk skew: the tensor queue never sits
        # behind a same-chunk exp/cast round trip
        s_phase(0)
        for lc in range(1, NLC):
            s_phase(lc)
            tp_phase(lc - 1)
        nc.scalar.dma_start(S_d.ap(), ssum[:])
        tp_phase(NLC - 1)

    nc.compile()
    return nc


def kernel(H, G, Wq_core, Wk_core, Wq_win, Wk_win, attn_mask):
    H = np.asarray(H, dtype=np.float32)
    G = np.asarray(G, dtype=np.float32)
    Wq_core = np.asarray(Wq_core, dtype=np.float32)
    Wk_core = np.asarray(Wk_core, dtype=np.float32)
    Wq_win = np.asarray(Wq_win, dtype=np.float32)
    Wk_win = np.asarray(Wk_win, dtype=np.float32)
    mask = np.asarray(attn_mask).astype(bool)

    with_mask = not bool(mask.all())
    key = ("k", with_mask)
    if key not in _CACHE:
        _CACHE[key] = _build(with_mask)
    nc = _CACHE[key]

    # host-side tiny G projections (weight-space only, no H involvement)
    qk = (G @ Wq_core) @ Wk_core.T * S_CORE          # [B, T, DH]
    qw2 = (G @ Wq_win) @ Wk_win.T * S_WIN            # [B, T, DH]
    selmat = np.tile(np.eye(32, dtype=np.float32), (4, 1))  # [128, 32]

    in_maps = []
    for c in range(8):
        b, half = c // 2, c % 2
        l0 = half * SPAN
        H16 = H[b, l0:l0 + SPAN, :].astype(np.float16)          # [2048, 1024]
        HT16 = np.ascontiguousarray(H[b].T[:, l0:l0 + SPAN]).astype(np.float16)
        # [i, p, lc, f] -> [lc, p, i, f]
        HTr = np.ascontiguousarray(
            HT16.reshape(8, 128, NLC, 512).transpose(2, 1, 0, 3)
        ).reshape(NLC, 128, 4096)
        # [j, s, p, f] -> [j, p, s, f]
        Hnr = np.ascontiguousarray(
            H16.reshape(NLC, 4, 128, DH).transpose(0, 2, 1, 3)
        ).reshape(NLC, 128, 4096)
        qkT16 = qk[b].T.astype(np.float16)                      # [1024, 32]
        QKTr = np.ascontiguousarray(
            qkT16.reshape(8, 128, 32).transpose(1, 0, 2)
        ).reshape(128, 256)
        im = {"HT": HTr, "Hn": Hnr, "QKT": QKTr, "SEL": selmat}
        if with_mask:
            im["maskbias"] = np.where(mask[b, l0:l0 + SPAN], 0.0,
                                      -1e9).astype(np.float32)[None, :]
            im["onesrow"] = np.ones((1, T), dtype=np.float32)
        in_maps.append(im)

    import os
    prof_dir = os.environ.get("BGW_PROFILE_DIR")
    res = None
    if prof_dir:
        try:
            res = run_bass_kernel_spmd(nc, in_maps, core_ids=list(range(8)),
                                       trace=True, tmpdir=prof_dir)
        except Exception:
            res = None
    if res is None:
        res = run_bass_kernel_spmd(nc, in_maps, core_ids=list(range(8)))
    kernel._last_result = res

    # ---- host combine: windows = sums of 4 chunk partials, tiny softmax ----
    NCHB = L // 256                                   # 16 chunks per batch
    Z = np.empty((B, T, DH), dtype=np.float32)
    for b in range(B):
        P = np.empty((NCHB, T, DH), dtype=np.float32)
        ss = np.empty((NCHB, T), dtype=np.float32)
        for half in range(2):
            r = res.results[2 * b + half]
            arr = r["P_out"].astype(np.float32).reshape(4, 32, NLC, 512)
            for lc in range(NLC):
                for cc in range(2):
                    for h in range(2):
                        P[half * NCH + 2 * lc + cc, :, h * 512:(h + 1) * 512] = \
                            arr[2 * cc + h, :, lc, :]
            ss[half * NCH:(half + 1) * NCH] = r["ssum_out"].T
        Zw = np.empty((W, T, DH), dtype=np.float32)
        wlog = np.empty((T, W), dtype=np.float32)
        for w in range(W):
            num = P[w] + P[w + 1] + P[w + 2] + P[w + 3]
            den = ss[w] + ss[w + 1] + ss[w + 2] + ss[w + 3]
            Zw[w] = num / den[:, None]
            wlog[:, w] = (Zw[w] * qw2[b]).sum(-1)
        m2 = wlog.max(-1, keepdims=True)
        wsm = np.exp(wlog - m2)
        wsm /= wsm.sum(-1, keepdims=True)
        Z[b] = np.einsum("tw,wtd->td", wsm, Zw)
    return Z


# revision 9
# speedup vs baseline: 3.4512x; 1.1167x over previous
"""Trainium2 Bass kernel for nn_BucketedGoWatti (sparse windowed attention pooling).

Math (B=4, L=4096, T=32, DH=1024, DG=256, DP=256, WIN=1024, STRIDE=256, W=13):
  All 13 windows are unions of 4 consecutive 256-wide l-chunks, and the
  per-window logits are slices of one global score matrix
    S[b,t,l] = (qk_b @ H_b^T)[t,l],  qk = (G Wq_core) Wk_core^T * DP^-0.5.
  So the device only computes, per 256-chunk c:
    E = exp(S),  s_c[t] = sum_{l in c} E[t,l],  P_c[t,:] = E[t, c] @ H[c, :]
  and the tiny 13-window combine (Zw = sum4(P)/sum4(s), cross-window softmax
  with qw2 = (G Wq_win) Wk_win^T * DH^-0.5) runs on host in f32.

Sharding: core = 2*b + half owns l in [half*2048, half*2048+2048) of batch b
(8 disjoint 256-chunks). H ships in fp16; the transposed layout needed by S
is only partially shipped (HBM is the bottleneck): chunk 0's H^T is built
fully on-chip by PE transposes from the natural tiles, chunks 1-2 half, and
chunk 3 ships whole so the last-arrival tail stays short.

PE shape: T=32 output rows -> 4x column tiling, so S and P run as 4
concurrent matmuls; the cross-col-group S reduce is one f32r matmul against
a [128,32] stacked identity. Numpy-sim rel err ~4e-4 (gate 2e-2).
"""
import numpy as np
from contextlib import ExitStack

import concourse.bacc as bacc
import concourse.tile as tile
import concourse.mybir as mybir
import concourse.masks as masks
from concourse.bass_utils import run_bass_kernel_spmd

F32 = mybir.dt.float32
F32R = mybir.dt.float32r
F16 = mybir.dt.float16
ActFn = mybir.ActivationFunctionType

B, L, T = 4, 4096, 32
DH, DG, DP = 1024, 256, 256
WIN, STRIDE = 1024, 256
W = (L - WIN) // STRIDE + 1          # 13
SPAN = 2048                          # per-core l-span
NLC = SPAN // 512                    # 4 l-chunks of 512
NCH = SPAN // 256                    # 8 chunks of 256 (the window quanta)
S_CORE = 1.0 / float(np.sqrt(DP))
S_WIN = 1.0 / float(np.sqrt(DH))

_CACHE = {}


def _build(with_mask: bool):
    nc = bacc.Bacc("TRN2", debug=False, target_bir_lowering=False)

    # natural layout [j, p, s*1024+d] with l = j*512 + s*128 + p
    Hn_d = nc.dram_tensor("Hn", [NLC, 128, 4096], F16, kind="ExternalInput")
    # shipped transposed parts: lc 1,2 d-tiles 0-3; lc 3 all 8
    HTq_d = nc.dram_tensor("HTq", [2, 128, 2048], F16, kind="ExternalInput")
    HT8_d = nc.dram_tensor("HT8", [128, 4096], F16, kind="ExternalInput")
    # [p, i*32+t] = qkT[i*128+p, t]
    QKT_d = nc.dram_tensor("QKT", [128, 256], F16, kind="ExternalInput")
    SEL_d = nc.dram_tensor("SEL", [128, 32], F32R, kind="ExternalInput")
    if with_mask:
        mb_d = nc.dram_tensor("maskbias", [1, SPAN], F32R, kind="ExternalInput")
        ones_d = nc.dram_tensor("onesrow", [1, T], F32R, kind="ExternalInput")
    # row 32*(2*cc+h)+t, col lc*512+d'  ->  P[2lc+cc][t, h*512+d']
    P_d = nc.dram_tensor("P_out", [128, NLC * 512], F16, kind="ExternalOutput")
    S_d = nc.dram_tensor("ssum_out", [T, NCH], F32, kind="ExternalOutput")

    with tile.TileContext(nc) as tc, ExitStack() as ctx:
        const = ctx.enter_context(tc.tile_pool(name="const", bufs=1))
        hpool = ctx.enter_context(tc.tile_pool(name="hpool", bufs=4))
        htqp = ctx.enter_context(tc.tile_pool(name="htqp", bufs=2))
        hth0p = ctx.enter_context(tc.tile_pool(name="hth0p", bufs=1))
        hthp = ctx.enter_context(tc.tile_pool(name="hthp", bufs=2))
        ep = ctx.enter_context(tc.tile_pool(name="ep", bufs=2))
        spl = ctx.enter_context(tc.tile_pool(name="spl", bufs=2))
        etp = ctx.enter_context(tc.tile_pool(name="etp", bufs=2))
        sb = ctx.enter_context(tc.tile_pool(name="sb", bufs=1))
        sp4 = ctx.enter_context(tc.tile_pool(name="sp4", bufs=2, space="PSUM"))
        lg = ctx.enter_context(tc.tile_pool(name="lg", bufs=2, space="PSUM"))
        pj = ctx.enter_context(tc.tile_pool(name="pj", bufs=2, space="PSUM"))
        zp = ctx.enter_context(tc.tile_pool(name="zp", bufs=2, space="PSUM"))

        ident = const.tile([128, 128], F32, tag="ident")
        masks.make_identity(nc, ident[:])
        identh = const.tile([128, 128], F16, tag="identh")
        nc.vector.tensor_copy(identh[:], ident[:])

        sel = const.tile([128, 32], F32R, tag="sel")
        nc.scalar.dma_start(sel[:], SEL_d.ap())
        qkt = const.tile([128, 256], F16, tag="qkt")
        nc.scalar.dma_start(qkt[:], QKT_d.ap())
        if with_mask:
            mbias = const.tile([1, SPAN], F32R, tag="mbias")
            onesr = const.tile([1, T], F32R, tag="onesr")
            nc.gpsimd.dma_start(mbias[:], mb_d.ap())
            nc.gpsimd.dma_start(onesr[:], ones_d.ap())

        # big input stream, ordered by first need
        hn = []
        for lc in range(NLC):
            t2_ = hpool.tile([128, 4096], F16, tag="hn")
            hn.append(t2_)
        htq = {}
        nc.sync.dma_start(hn[0][:], Hn_d.ap()[0])
        nc.sync.dma_start(hn[1][:], Hn_d.ap()[1])
        for lc in (1, 2):
            t_ = htqp.tile([128, 2048], F16, tag="htq")
            nc.sync.dma_start(t_[:], HTq_d.ap()[lc - 1])
            htq[lc] = t_
            if lc == 1:
                nc.sync.dma_start(hn[2][:], Hn_d.ap()[2])
        ht8 = const.tile([128, 4096], F16, tag="ht8")
        nc.sync.dma_start(ht8[:], HT8_d.ap())
        nc.sync.dma_start(hn[3][:], Hn_d.ap()[3])

        # ~4us of dummy f32 matmuls while the first DMAs stream in: trips the
        # PE HAM un-throttle (~3.4us sustained busy) so later matmuls run at
        # 2.4GHz instead of the cold 1.2GHz default.
        warm = sp4.tile([128, 512], F32, tag="spart")
        for _ in range(9):
            nc.tensor.matmul(warm[:, :128], ident[:], ident[:],
                             start=True, stop=True)

        hth = {}

        def h_tr(lc, i0, n, dst):
            # build HT d-tiles [i0, i0+n) on-chip from natural tiles
            for ii in range(i0, i0 + n, 2):
                pst = pj.tile([128, 1024], F16, tag="pj")
                for m in range(2):
                    i = ii + m
                    for s in range(4):
                        nc.tensor.transpose(
                            pst[:, (m * 4 + s) * 128:(m * 4 + s + 1) * 128],
                            hn[lc][:, s * 1024 + i * 128:s * 1024 + (i + 1) * 128],
                            identh[:])
                nc.vector.tensor_copy(
                    dst[:, (ii - i0) * 512:(ii - i0 + 2) * 512], pst[:])

        def s_rhs(lc, i):
            if lc == 0:
                return hth[0][:, i * 512:(i + 1) * 512]
            if lc == 3:
                return ht8[:, i * 512:(i + 1) * 512]
            if i < 4:
                return htq[lc][:, i * 512:(i + 1) * 512]
            return hth[lc][:, (i - 4) * 512:(i - 3) * 512]

        pout = sb.tile([128, NLC * 512], F16, tag="pout")
        ssum = sb.tile([T, NCH], F32, tag="ssum")
        es = {}

        def s_phase(lc):
            if lc == 0:
                hth[0] = hth0p.tile([128, 4096], F16, tag="hth0", name="hth0")
                h_tr(0, 0, 8, hth[0])
            elif lc < 3:
                hth[lc] = hthp.tile([128, 2048], F16, tag="hth", name="hth")
                h_tr(lc, 4, 4, hth[lc])
            # --- S[t, l]: 4 col-groups x 2 d-tiles each ---
            spart = sp4.tile([128, 512], F32, tag="spart")
            for k in range(2):
                for g in range(4):
                    i = g + 4 * k
                    nc.tensor.matmul(spart[32 * g:32 * g + 32, :],
                                     qkt[:, i * 32:(i + 1) * 32], s_rhs(lc, i),
                                     start=(k == 0), stop=(k == 1),
                                     tile_position=(0, 32 * g))
            sparts = spl.tile([128, 512], F32R, tag="sps")
            nc.vector.tensor_copy(sparts[:], spart[:])
            sps = lg.tile([T, 512], F32, tag="lg")
            nc.tensor.matmul(sps[:], sel[:], sparts[:],
                             start=True, stop=(not with_mask))
            if with_mask:
                nc.tensor.matmul(sps[:], onesr[:],
                                 mbias[:, lc * 512:(lc + 1) * 512],
                                 start=False, stop=True)
            # --- E = exp(S) in fp16, with per-256-chunk row sums ---
            e_ = ep.tile([T, 512], F16, tag="e")
            for u in range(2):
                c = 2 * lc + u
                nc.scalar.activation(e_[:, u * 256:(u + 1) * 256],
                                     sps[:, u * 256:(u + 1) * 256],
                                     ActFn.Exp, accum_out=ssum[:, c:c + 1])
            es[lc] = e_

        def tp_phase(lc):
            # --- E^T fp16: 4 PE transposes packed in one PSUM tile, 1 copy ---
            e_ = es[lc]
            pst = pj.tile([128, 1024], F16, tag="pj")
            for k in range(4):
                nc.tensor.transpose(pst[:, k * 32:(k + 1) * 32],
                                    e_[:, k * 128:(k + 1) * 128],
                                    identh[:32, :32])
            et = etp.tile([128, 128], F16, tag="et")
            nc.vector.tensor_copy(et[:], pst[:, :128])
            # --- P: 4 col-groups j=(cc,h), 2 l-subtiles each ---
            ppack = zp.tile([128, 512], F32, tag="zp")
            for k in range(2):
                for j in range(4):
                    cc, h = j // 2, j % 2
                    s = 2 * cc + k
                    nc.tensor.matmul(ppack[32 * j:32 * j + 32, :],
                                     et[:, s * 32:(s + 1) * 32],
                                     hn[lc][:, s * 1024 + h * 512:
                                            s * 1024 + h * 512 + 512],
                                     start=(k == 0), stop=(k == 1),
                                     tile_position=(0, 32 * j))
            if lc in (0, 2):
                nc.scalar.activation(pout[:, lc * 512:(lc + 1) * 512], ppack[:],
                                     ActFn.Identity)
            else:
                nc.vector.tensor_copy(pout[:, lc * 512:(lc + 1) * 512], ppack[:])
            # ship this l-chunk's quarter of P as soon as its copy lands
            nc.scalar.dma_start(P_d.ap()[:, lc * 512:(lc + 1) * 512],
                                pout[:, lc * 512:(lc + 1) * 512])

        s_phase(0)
        for lc in range(1, NLC):
            s_phase(lc)
            tp_phase(lc - 1)
        nc.scalar.dma_start(S_d.ap(), ssum[:])
        tp_phase(NLC - 1)

    nc.compile()
    return nc


def kernel(H, G, Wq_core, Wk_core, Wq_win, Wk_win, attn_mask):
    H = np.asarray(H, dtype=np.float32)
    G = np.asarray(G, dtype=np.float32)
    Wq_core = np.asarray(Wq_core, dtype=np.float32)
    Wk_core = np.asarray(Wk_core, dtype=np.float32)
    Wq_win = np.asarray(Wq_win, dtype=np.float32)
    Wk_win = np.asarray(Wk_win, dtype=np.float32)
    mask = np.asarray(attn_mask).astype(bool)

    with_mask = not bool(mask.all())
    key = ("k", with_mask)
    if key not in _CACHE:
        _CACHE[key] = _build(with_mask)
    nc = _CACHE[key]

    # host-side tiny G projections (weight-space only, no H involvement)
    qk = (G @ Wq_core) @ Wk_core.T * S_CORE          # [B, T, DH]
    qw2 = (G @ Wq_win) @ Wk_win.T * S_WIN            # [B, T, DH]
    selmat = np.tile(np.eye(32, dtype=np.float32), (4, 1))  # [128, 32]

    in_maps = []
    for c in range(8):
        b, half = c // 2, c % 2
        l0 = half * SPAN
        H16 = H[b, l0:l0 + SPAN, :].astype(np.float16)          # [2048, 1024]
        HT16 = np.ascontiguousarray(H[b].T[:, l0:l0 + SPAN]).astype(np.float16)
        # [i, p, lc, f] -> [lc, p, i, f]
        HTr = np.ascontiguousarray(
            HT16.reshape(8, 128, NLC, 512).transpose(2, 1, 0, 3)
        ).reshape(NLC, 128, 4096)
        # [j, s, p, f] -> [j, p, s, f]
        Hnr = np.ascontiguousarray(
            H16.reshape(NLC, 4, 128, DH).transpose(0, 2, 1, 3)
        ).reshape(NLC, 128, 4096)
        qkT16 = qk[b].T.astype(np.float16)                      # [1024, 32]
        QKTr = np.ascontiguousarray(
            qkT16.reshape(8, 128, 32).transpose(1, 0, 2)
        ).reshape(128, 256)
        im = {"Hn": Hnr, "HTq": np.ascontiguousarray(HTr[1:3, :, :2048]),
              "HT8": HTr[3], "QKT": QKTr, "SEL": selmat}
        if with_mask:
            im["maskbias"] = np.where(mask[b, l0:l0 + SPAN], 0.0,
                                      -1e9).astype(np.float32)[None, :]
            im["onesrow"] = np.ones((1, T), dtype=np.float32)
        in_maps.append(im)

    import os
    prof_dir = os.environ.get("BGW_PROFILE_DIR")
    res = None
    if prof_dir:
        try:
            res = run_bass_kernel_spmd(nc, in_maps, core_ids=list(range(8)),
                                       trace=True, tmpdir=prof_dir)
        except Exception:
            res = None
    if res is None:
        res = run_bass_kernel_spmd(nc, in_maps, core_ids=list(range(8)))
    kernel._last_result = res

    # ---- host combine: windows = sums of 4 chunk partials, tiny softmax ----
    NCHB = L // 256                                   # 16 chunks per batch
    Z = np.empty((B, T, DH), dtype=np.float32)
    for b in range(B):
        P = np.empty((NCHB, T, DH), dtype=np.float32)
        ss = np.empty((NCHB, T), dtype=np.float32)
        for half in range(2):
            r = res.results[2 * b + half]
            arr = r["P_out"].astype(np.float32).reshape(4, 32, NLC, 512)
            for lc in range(NLC):
                for cc in range(2):
                    for h in range(2):
                        P[half * NCH + 2 * lc + cc, :, h * 512:(h + 1) * 512] = \
                            arr[2 * cc + h, :, lc, :]
            ss[half * NCH:(half + 1) * NCH] = r["ssum_out"].T
        Zw = np.empty((W, T, DH), dtype=np.float32)
        wlog = np.empty((T, W), dtype=np.float32)
        for w in range(W):
            num = P[w] + P[w + 1] + P[w + 2] + P[w + 3]
            den = ss[w] + ss[w + 1] + ss[w + 2] + ss[w + 3]
            Zw[w] = num / den[:, None]
            wlog[:, w] = (Zw[w] * qw2[b]).sum(-1)
        m2 = wlog.max(-1, keepdims=True)
        wsm = np.exp(wlog - m2)
        wsm /= wsm.sum(-1, keepdims=True)
        Z[b] = np.einsum("tw,wtd->td", wsm, Zw)
    return Z


# revision 10
# speedup vs baseline: 3.4593x; 1.0023x over previous
"""Trainium2 Bass kernel for nn_BucketedGoWatti (sparse windowed attention pooling).

Math (B=4, L=4096, T=32, DH=1024, DG=256, DP=256, WIN=1024, STRIDE=256, W=13):
  All 13 windows are unions of 4 consecutive 256-wide l-chunks, and the
  per-window logits are slices of one global score matrix
    S[b,t,l] = (qk_b @ H_b^T)[t,l],  qk = (G Wq_core) Wk_core^T * DP^-0.5.
  So the device only computes, per 256-chunk c:
    E = exp(S),  s_c[t] = sum_{l in c} E[t,l],  P_c[t,:] = E[t, c] @ H[c, :]
  and the tiny 13-window combine (Zw = sum4(P)/sum4(s), cross-window softmax
  with qw2 = (G Wq_win) Wk_win^T * DH^-0.5) runs on host in f32.

Sharding: core = 2*b + half owns l in [half*2048, half*2048+2048) of batch b
(8 disjoint 256-chunks). H ships in fp16; the transposed layout needed by S
is only partially shipped (HBM is the bottleneck): chunk 0's H^T is built
fully on-chip by PE transposes from the natural tiles, chunks 1-2 half, and
chunk 3 ships whole so the last-arrival tail stays short.

PE shape: T=32 output rows -> 4x column tiling, so S and P run as 4
concurrent matmuls; the cross-col-group S reduce is one f32r matmul against
a [128,32] stacked identity. Numpy-sim rel err ~4e-4 (gate 2e-2).
"""
import numpy as np
from contextlib import ExitStack

import concourse.bacc as bacc
import concourse.tile as tile
import concourse.mybir as mybir
import concourse.masks as masks
from concourse.bass_utils import run_bass_kernel_spmd

F32 = mybir.dt.float32
F32R = mybir.dt.float32r
F16 = mybir.dt.float16
ActFn = mybir.ActivationFunctionType

B, L, T = 4, 4096, 32
DH, DG, DP = 1024, 256, 256
WIN, STRIDE = 1024, 256
W = (L - WIN) // STRIDE + 1          # 13
SPAN = 2048                          # per-core l-span
NLC = SPAN // 512                    # 4 l-chunks of 512
NCH = SPAN // 256                    # 8 chunks of 256 (the window quanta)
S_CORE = 1.0 / float(np.sqrt(DP))
S_WIN = 1.0 / float(np.sqrt(DH))

_CACHE = {}


def _build(with_mask: bool):
    nc = bacc.Bacc("TRN2", debug=False, target_bir_lowering=False)

    # natural layout [j, p, s*1024+d] with l = j*512 + s*128 + p
    Hn_d = nc.dram_tensor("Hn", [NLC, 128, 4096], F16, kind="ExternalInput")
    # shipped transposed parts: lc 1,2 d-tiles 0-3; lc 3 all 8
    HTq_d = nc.dram_tensor("HTq", [1, 128, 2048], F16, kind="ExternalInput")
    HT8_d = nc.dram_tensor("HT8", [128, 4096], F16, kind="ExternalInput")
    # [p, i*32+t] = qkT[i*128+p, t]
    QKT_d = nc.dram_tensor("QKT", [128, 256], F16, kind="ExternalInput")
    SEL_d = nc.dram_tensor("SEL", [128, 32], F32R, kind="ExternalInput")
    if with_mask:
        mb_d = nc.dram_tensor("maskbias", [1, SPAN], F32R, kind="ExternalInput")
        ones_d = nc.dram_tensor("onesrow", [1, T], F32R, kind="ExternalInput")
    # row 32*(2*cc+h)+t, col lc*512+d'  ->  P[2lc+cc][t, h*512+d']
    P_d = nc.dram_tensor("P_out", [128, NLC * 512], F16, kind="ExternalOutput")
    S_d = nc.dram_tensor("ssum_out", [T, NCH], F32, kind="ExternalOutput")

    with tile.TileContext(nc) as tc, ExitStack() as ctx:
        const = ctx.enter_context(tc.tile_pool(name="const", bufs=1))
        hpool = ctx.enter_context(tc.tile_pool(name="hpool", bufs=4))
        htqp = ctx.enter_context(tc.tile_pool(name="htqp", bufs=2))
        hthf = ctx.enter_context(tc.tile_pool(name="hthf", bufs=2))
        hthp = ctx.enter_context(tc.tile_pool(name="hthp", bufs=1))
        ep = ctx.enter_context(tc.tile_pool(name="ep", bufs=2))
        spl = ctx.enter_context(tc.tile_pool(name="spl", bufs=2))
        etp = ctx.enter_context(tc.tile_pool(name="etp", bufs=2))
        sb = ctx.enter_context(tc.tile_pool(name="sb", bufs=1))
        sp4 = ctx.enter_context(tc.tile_pool(name="sp4", bufs=2, space="PSUM"))
        lg = ctx.enter_context(tc.tile_pool(name="lg", bufs=2, space="PSUM"))
        pj = ctx.enter_context(tc.tile_pool(name="pj", bufs=2, space="PSUM"))
        zp = ctx.enter_context(tc.tile_pool(name="zp", bufs=2, space="PSUM"))

        ident = const.tile([128, 128], F32, tag="ident")
        masks.make_identity(nc, ident[:])
        identh = const.tile([128, 128], F16, tag="identh")
        nc.vector.tensor_copy(identh[:], ident[:])

        sel = const.tile([128, 32], F32R, tag="sel")
        nc.scalar.dma_start(sel[:], SEL_d.ap())
        qkt = const.tile([128, 256], F16, tag="qkt")
        nc.scalar.dma_start(qkt[:], QKT_d.ap())
        if with_mask:
            mbias = const.tile([1, SPAN], F32R, tag="mbias")
            onesr = const.tile([1, T], F32R, tag="onesr")
            nc.gpsimd.dma_start(mbias[:], mb_d.ap())
            nc.gpsimd.dma_start(onesr[:], ones_d.ap())

        # big input stream, ordered by first need
        hn = []
        for lc in range(NLC):
            t2_ = hpool.tile([128, 4096], F16, tag="hn")
            hn.append(t2_)
        htq = {}
        nc.sync.dma_start(hn[0][:], Hn_d.ap()[0])
        nc.sync.dma_start(hn[1][:], Hn_d.ap()[1])
        ht8 = const.tile([128, 4096], F16, tag="ht8")
        nc.sync.dma_start(ht8[:], HT8_d.ap())
        nc.sync.dma_start(hn[2][:], Hn_d.ap()[2])
        t_ = htqp.tile([128, 2048], F16, tag="htq")
        nc.sync.dma_start(t_[:], HTq_d.ap()[0])
        htq[2] = t_
        nc.sync.dma_start(hn[3][:], Hn_d.ap()[3])

        # ~4us of dummy f32 matmuls while the first DMAs stream in: trips the
        # PE HAM un-throttle (~3.4us sustained busy) so later matmuls run at
        # 2.4GHz instead of the cold 1.2GHz default.
        warm = sp4.tile([128, 512], F32, tag="spart")
        for _ in range(12):
            nc.tensor.matmul(warm[:, :128], ident[:], ident[:],
                             start=True, stop=True)

        hth = {}

        def h_tr(lc, i0, n, dst):
            # build HT d-tiles [i0, i0+n) on-chip from natural tiles
            for ii in range(i0, i0 + n, 2):
                pst = pj.tile([128, 1024], F16, tag="pj")
                for m in range(2):
                    i = ii + m
                    for s in range(4):
                        nc.tensor.transpose(
                            pst[:, (m * 4 + s) * 128:(m * 4 + s + 1) * 128],
                            hn[lc][:, s * 1024 + i * 128:s * 1024 + (i + 1) * 128],
                            identh[:])
                nc.vector.tensor_copy(
                    dst[:, (ii - i0) * 512:(ii - i0 + 2) * 512], pst[:])

        def s_rhs(lc, i):
            if lc in (0, 1):
                return hth[lc][:, i * 512:(i + 1) * 512]
            if lc == 3:
                return ht8[:, i * 512:(i + 1) * 512]
            if i < 4:
                return htq[lc][:, i * 512:(i + 1) * 512]
            return hth[lc][:, (i - 4) * 512:(i - 3) * 512]

        pout = sb.tile([128, NLC * 512], F16, tag="pout")
        ssum = sb.tile([T, NCH], F32, tag="ssum")
        es = {}

        def s_phase(lc):
            if lc in (0, 1):
                hth[lc] = hthf.tile([128, 4096], F16, tag="hthf", name="hthf")
                h_tr(lc, 0, 8, hth[lc])
            elif lc == 2:
                hth[lc] = hthp.tile([128, 2048], F16, tag="hth", name="hth")
                h_tr(lc, 4, 4, hth[lc])
            # --- S[t, l]: 4 col-groups x 2 d-tiles each ---
            spart = sp4.tile([128, 512], F32, tag="spart")
            for k in range(2):
                for g in range(4):
                    i = g + 4 * k
                    nc.tensor.matmul(spart[32 * g:32 * g + 32, :],
                                     qkt[:, i * 32:(i + 1) * 32], s_rhs(lc, i),
                                     start=(k == 0), stop=(k == 1),
                                     tile_position=(0, 32 * g))
            sparts = spl.tile([128, 512], F32R, tag="sps")
            nc.vector.tensor_copy(sparts[:], spart[:])
            sps = lg.tile([T, 512], F32, tag="lg")
            nc.tensor.matmul(sps[:], sel[:], sparts[:],
                             start=True, stop=(not with_mask))
            if with_mask:
                nc.tensor.matmul(sps[:], onesr[:],
                                 mbias[:, lc * 512:(lc + 1) * 512],
                                 start=False, stop=True)
            # --- E = exp(S) in fp16, with per-256-chunk row sums ---
            e_ = ep.tile([T, 512], F16, tag="e")
            nc.scalar.activation(e_[:], sps[:], ActFn.Exp)
            for u in range(2):
                c = 2 * lc + u
                nc.vector.reduce_sum(ssum[:, c:c + 1],
                                     e_[:, u * 256:(u + 1) * 256],
                                     axis=mybir.AxisListType.X)
            es[lc] = e_

        def tp_phase(lc):
            # --- E^T fp16: 4 PE transposes packed in one PSUM tile, 1 copy ---
            e_ = es[lc]
            pst = pj.tile([128, 1024], F16, tag="pj")
            for k in range(4):
                nc.tensor.transpose(pst[:, k * 32:(k + 1) * 32],
                                    e_[:, k * 128:(k + 1) * 128],
                                    identh[:32, :32])
            et = etp.tile([128, 128], F16, tag="et")
            nc.vector.tensor_copy(et[:], pst[:, :128])
            # --- P: 4 col-groups j=(cc,h), 2 l-subtiles each ---
            ppack = zp.tile([128, 512], F32, tag="zp")
            for k in range(2):
                for j in range(4):
                    cc, h = j // 2, j % 2
                    s = 2 * cc + k
                    nc.tensor.matmul(ppack[32 * j:32 * j + 32, :],
                                     et[:, s * 32:(s + 1) * 32],
                                     hn[lc][:, s * 1024 + h * 512:
                                            s * 1024 + h * 512 + 512],
                                     start=(k == 0), stop=(k == 1),
                                     tile_position=(0, 32 * j))
            if lc < 3:
                nc.scalar.activation(pout[:, lc * 512:(lc + 1) * 512], ppack[:],
                                     ActFn.Identity)
            else:
                nc.vector.tensor_copy(pout[:, lc * 512:(lc + 1) * 512], ppack[:])
            # ship this l-chunk's quarter of P as soon as its copy lands
            nc.scalar.dma_start(P_d.ap()[:, lc * 512:(lc + 1) * 512],
                                pout[:, lc * 512:(lc + 1) * 512])

        s_phase(0)
        s_phase(1)
        tp_phase(0)
        s_phase(3)
        tp_phase(1)
        s_phase(2)
        tp_phase(2)
        nc.scalar.dma_start(S_d.ap(), ssum[:])
        tp_phase(3)

    nc.compile()
    return nc


def kernel(H, G, Wq_core, Wk_core, Wq_win, Wk_win, attn_mask):
    H = np.asarray(H, dtype=np.float32)
    G = np.asarray(G, dtype=np.float32)
    Wq_core = np.asarray(Wq_core, dtype=np.float32)
    Wk_core = np.asarray(Wk_core, dtype=np.float32)
    Wq_win = np.asarray(Wq_win, dtype=np.float32)
    Wk_win = np.asarray(Wk_win, dtype=np.float32)
    mask = np.asarray(attn_mask).astype(bool)

    with_mask = not bool(mask.all())
    key = ("k", with_mask)
    if key not in _CACHE:
        _CACHE[key] = _build(with_mask)
    nc = _CACHE[key]

    # host-side tiny G projections (weight-space only, no H involvement)
    qk = (G @ Wq_core) @ Wk_core.T * S_CORE          # [B, T, DH]
    qw2 = (G @ Wq_win) @ Wk_win.T * S_WIN            # [B, T, DH]
    selmat = np.tile(np.eye(32, dtype=np.float32), (4, 1))  # [128, 32]

    in_maps = []
    for c in range(8):
        b, half = c // 2, c % 2
        l0 = half * SPAN
        H16 = H[b, l0:l0 + SPAN, :].astype(np.float16)          # [2048, 1024]
        HT16 = np.ascontiguousarray(H[b].T[:, l0:l0 + SPAN]).astype(np.float16)
        # [i, p, lc, f] -> [lc, p, i, f]
        HTr = np.ascontiguousarray(
            HT16.reshape(8, 128, NLC, 512).transpose(2, 1, 0, 3)
        ).reshape(NLC, 128, 4096)
        # [j, s, p, f] -> [j, p, s, f]
        Hnr = np.ascontiguousarray(
            H16.reshape(NLC, 4, 128, DH).transpose(0, 2, 1, 3)
        ).reshape(NLC, 128, 4096)
        qkT16 = qk[b].T.astype(np.float16)                      # [1024, 32]
        QKTr = np.ascontiguousarray(
            qkT16.reshape(8, 128, 32).transpose(1, 0, 2)
        ).reshape(128, 256)
        im = {"Hn": Hnr, "HTq": np.ascontiguousarray(HTr[2:3, :, :2048]),
              "HT8": HTr[3], "QKT": QKTr, "SEL": selmat}
        if with_mask:
            im["maskbias"] = np.where(mask[b, l0:l0 + SPAN], 0.0,
                                      -1e9).astype(np.float32)[None, :]
            im["onesrow"] = np.ones((1, T), dtype=np.float32)
        in_maps.append(im)

    import os
    prof_dir = os.environ.get("BGW_PROFILE_DIR")
    res = None
    if prof_dir:
        try:
            res = run_bass_kernel_spmd(nc, in_maps, core_ids=list(range(8)),
                                       trace=True, tmpdir=prof_dir)
        except Exception:
            res = None
    if res is None:
        res = run_bass_kernel_spmd(nc, in_maps, core_ids=list(range(8)))
    kernel._last_result = res

    # ---- host combine: windows = sums of 4 chunk partials, tiny softmax ----
    NCHB = L // 256                                   # 16 chunks per batch
    Z = np.empty((B, T, DH), dtype=np.float32)
    for b in range(B):
        P = np.empty((NCHB, T, DH), dtype=np.float32)
        ss = np.empty((NCHB, T), dtype=np.float32)
        for half in range(2):
            r = res.results[2 * b + half]
            arr = r["P_out"].astype(np.float32).reshape(4, 32, NLC, 512)
            for lc in range(NLC):
                for cc in range(2):
                    for h in range(2):
                        P[half * NCH + 2 * lc + cc, :, h * 512:(h + 1) * 512] = \
                            arr[2 * cc + h, :, lc, :]
            ss[half * NCH:(half + 1) * NCH] = r["ssum_out"].T
        Zw = np.empty((W, T, DH), dtype=np.float32)
        wlog = np.empty((T, W), dtype=np.float32)
        for w in range(W):
            num = P[w] + P[w + 1] + P[w + 2] + P[w + 3]
            den = ss[w] + ss[w + 1] + ss[w + 2] + ss[w + 3]
            Zw[w] = num / den[:, None]
            wlog[:, w] = (Zw[w] * qw2[b]).sum(-1)
        m2 = wlog.max(-1, keepdims=True)
        wsm = np.exp(wlog - m2)
        wsm /= wsm.sum(-1, keepdims=True)
        Z[b] = np.einsum("tw,wtd->td", wsm, Zw)
    return Z
